# revision 19
# baseline (speedup 1.0000x reference)
"""Masked self-attention Trainium2 kernel.

Reference computes (per batch b):
    key   = x @ Wk.T            [S, 64]
    query = x @ Wq.T            [S, 64]
    value = x @ Wv.T            [S, 128]
    kT_m  = tril(key.T)         [64, S]   -- element (d, s) kept iff s <= d
    out   = softmax(query @ kT_m, axis=-1) @ value

Because kT_m's tril zeroes every column s >= 64, score[i, s] = 0 for all
s >= 64 and score[i, s] = sum_{d>=s} q[i,d] k[s,d] for s < 64.  With a fixed
stability shift c (exactly equivalent to softmax's max-subtraction with m=c):

    out[i] = (sum_{s<64} e^{z_s - c} v[s]  +  e^{-c} * Vtail) /
             (sum_{s<64} e^{z_s - c}       +  e^{-c} * (S-64))

where Vtail = sum_{s>=64} value[s] = (sum_{s>=64} x[s]) @ Wv.T (linearity).
z stays within about +-55 for these inputs, so c=20 keeps every exp inside
fp32 range and preserves relative precision identically to max-subtraction.

Per-core computation (8 cores; core = (batch b, half h), 2048 tokens each):
    zT   = WzT.T @ xaT          with Wz = tril_mask(key64) @ Wq  (fused once)
    pT   = exp(zT - c),  augmented with a constant row e^{-c}
    oaug = pT.T @ [v64 | 1 ; Vtail | S-64]   -> numerator cols + denom col
    out  = oaug[:, :128] * (1 / oaug[:, 128])

Precision: inputs stream in as fp16 (single-pass PE matmuls, half the DMA
bytes); exp output, the value-side matmul, and the final output run in bf16
(p spans e^+-50, needing bf16's fp32-range exponent); every accumulation is
fp32 in PSUM.  Measured end-to-end relative error ~6e-3 vs fp32 reference.

Engine budget: each dma_start costs ~0.6us of sequencer time and ~5us
issue-to-completion latency, and each issuing engine (Sync/GpSimd/Scalar)
owns one hardware queue-set, so the big x load is split into four
partition-slices spread over all three queue-sets.  The batch column-sum
runs on DVE (own half) and the Scalar engine's activation accum_out
(other half) in parallel so the Vtail chain clears early.
"""

import numpy as np

import concourse.bass as bass
import concourse.bacc as bacc
import concourse.tile as tile
from concourse import mybir
from concourse.bass_utils import run_bass_kernel_spmd

F32 = mybir.dt.float32
F16 = mybir.dt.float16
BF16 = mybir.dt.bfloat16
AF = mybir.ActivationFunctionType
AX = mybir.AxisListType

B, S, E, KD = 4, 4096, 128, 64
HALF = S // 2            # tokens handled per core
NCORES = 8
CHUNK = 512              # tokens per z-matmul / exp (one PSUM bank)
NCHUNK = HALF // CHUNK
TSUB = 128               # tokens per output matmul (M <= 128)
NSUB = CHUNK // TSUB
CSHIFT = 20.0            # fixed softmax shift
P0 = float(np.exp(-CSHIFT))
NTAIL = float(S - KD)    # 4032 all-zero score columns

# wpack_e packs [x64T | wkT | wvT] on 128 partitions; wpack_q packs [wq | tri]
# on 64 partitions.  One DMA each instead of five.
X64_OFF, WK_OFF, WV_OFF = 0, KD, 2 * KD
WPE_COLS = 2 * KD + E
WQ_OFF, TRI_OFF = 0, E
WPQ_COLS = E + KD


def _build_nc() -> bass.Bass:
    nc = bacc.Bacc("TRN2", target_bir_lowering=False, debug=False)

    xfT = nc.dram_tensor("xfT", [E, S], F16, kind="ExternalInput").ap()
    wpe = nc.dram_tensor("wpe", [E, WPE_COLS], F16, kind="ExternalInput").ap()
    wpq = nc.dram_tensor("wpq", [KD, WPQ_COLS], F16, kind="ExternalInput").ap()
    out = nc.dram_tensor("out", [HALF, E], BF16, kind="ExternalOutput").ap()

    with tile.TileContext(nc) as tc:
        with (
            tc.tile_pool(name="singles", bufs=1) as singles,
            tc.tile_pool(name="pre_ps", bufs=1, space="PSUM") as pre_ps,
            tc.tile_pool(name="z_ps", bufs=2, space="PSUM") as z_ps,
            tc.tile_pool(name="o_ps", bufs=4, space="PSUM") as o_ps,
            tc.tile_pool(name="outp", bufs=4) as outp,
            tc.tile_pool(name="recs", bufs=4) as recs,
        ):
            # ---- DMA in.  Each issuing engine owns one HW queue-set, and
            # every dma_start pays ~5us issue-to-completion latency, so the
            # big x load is split across all three queue-sets and issued as
            # early as possible.
            wpe_sb = singles.tile([E, WPE_COLS], F16)
            nc.sync.dma_start(wpe_sb[:], wpe)
            wpq_sb = singles.tile([KD, WPQ_COLS], F16)
            nc.scalar.dma_start(wpq_sb[:], wpq)
            xfT_sb = singles.tile([E, S], F16)
            PSLC = E // 4
            for i, eng in enumerate((nc.sync, nc.scalar, nc.sync, nc.scalar)):
                ps = slice(i * PSLC, (i + 1) * PSLC)
                eng.dma_start(xfT_sb[ps, :], xfT[ps, :])

            x64T_sb = wpe_sb[:, X64_OFF : X64_OFF + KD]
            wkT_sb = wpe_sb[:, WK_OFF : WK_OFF + KD]
            wvT_sb = wpe_sb[:, WV_OFF : WV_OFF + E]
            wq_sb = wpq_sb[:, WQ_OFF : WQ_OFF + E]
            tri_sb = wpq_sb[:, TRI_OFF : TRI_OFF + KD]

            # ---- preamble ----
            # kT[d, s] = key64[s, d]
            kT_ps = pre_ps.tile([KD, KD], F32, tag="pre")
            nc.tensor.matmul(kT_ps[:], wkT_sb, x64T_sb, start=True, stop=True)
            kmT_sb = singles.tile([KD, KD], F16)
            nc.vector.tensor_mul(kmT_sb[:], kT_ps[:], tri_sb)

            # WzT[e, s] = sum_d Wq[d, e] km[s, d]
            wzT_ps = pre_ps.tile([E, KD], F32, tag="pre")
            nc.tensor.matmul(wzT_ps[:], wq_sb, kmT_sb[:], start=True, stop=True)
            wzT_sb = singles.tile([E, KD], F16)
            nc.vector.tensor_copy(wzT_sb[:], wzT_ps[:])

            # vaug = [[v64, 1], [vtail, NTAIL]] in bf16
            vaug_sb = singles.tile([KD + 1, E + 1], BF16)
            v64_ps = pre_ps.tile([KD, E], F32, tag="pre")
            nc.tensor.matmul(v64_ps[:], x64T_sb, wvT_sb, start=True, stop=True)
            nc.vector.tensor_copy(vaug_sb[0:KD, 0:E], v64_ps[:])
            nc.vector.memset(vaug_sb[0:KD, E : E + 1], 1.0)
            nc.vector.memset(vaug_sb[KD : KD + 1, E : E + 1], NTAIL)

            # tail column-sum of the batch: own half on DVE, other half on
            # the Scalar engine (activation accum_out), x64 correction on DVE.
            xsum_a = singles.tile([E, 1], F32)
            nc.vector.reduce_sum(out=xsum_a[:], in_=xfT_sb[:, 0:HALF], axis=AX.X)
            xsum_o = singles.tile([E, 1], F32)
            scratch_sb = singles.tile([E, HALF], F16)
            nc.scalar.activation(
                scratch_sb[:], xfT_sb[:, HALF:S], AF.Copy, accum_out=xsum_o[:]
            )
            xsum_64 = singles.tile([E, 1], F32)
            nc.vector.reduce_sum(out=xsum_64[:], in_=x64T_sb, axis=AX.X)
            xsum_all = singles.tile([E, 1], F32)
            nc.vector.tensor_add(xsum_all[:], xsum_a[:], xsum_o[:])
            xsum_h = singles.tile([E, 1], F16)
            nc.vector.tensor_sub(xsum_h[:], xsum_all[:], xsum_64[:])
            vtail_ps = pre_ps.tile([1, E], F32, tag="pre")
            nc.tensor.matmul(vtail_ps[:], xsum_h[:], wvT_sb, start=True, stop=True)
            nc.vector.tensor_copy(vaug_sb[KD : KD + 1, 0:E], vtail_ps[:])

            # ---- main loop ----
            nbias_sb = singles.tile([KD, 1], F32)
            nc.vector.memset(nbias_sb[:], -CSHIFT)
            pT_sb = singles.tile([KD + 1, HALF], BF16)
            nc.gpsimd.memset(pT_sb[KD : KD + 1, :], P0)
            out_engs = (nc.sync, nc.scalar, nc.sync, nc.scalar)
            for c in range(NCHUNK):
                cs = slice(c * CHUNK, (c + 1) * CHUNK)
                zT_ps = z_ps.tile([KD, CHUNK], F32, tag="z")
                nc.tensor.matmul(
                    zT_ps[:], wzT_sb[:], xfT_sb[:, cs], start=True, stop=True
                )
                nc.scalar.activation(
                    pT_sb[0:KD, cs], zT_ps[:], AF.Exp, bias=nbias_sb[:]
                )
                ob_sb = outp.tile([TSUB, NSUB, E], BF16, tag="ob")
                for t in range(NSUB):
                    tok = c * CHUNK + t * TSUB
                    oa_ps = o_ps.tile([TSUB, E + 1], F32, tag="oa")
                    nc.tensor.matmul(
                        oa_ps[:],
                        pT_sb[0 : KD + 1, tok : tok + TSUB],
                        vaug_sb[:],
                        start=True,
                        stop=True,
                    )
                    rec_sb = recs.tile([TSUB, 1], F32, tag="rec")
                    nc.vector.reciprocal(rec_sb[:], oa_ps[:, E : E + 1])
                    if t % 2 == 0:
                        nc.vector.tensor_scalar_mul(
                            ob_sb[:, t, :], oa_ps[:, 0:E], rec_sb[:]
                        )
                    else:
                        nc.scalar.activation(
                            ob_sb[:, t, :], oa_ps[:, 0:E], AF.Copy, scale=rec_sb[:]
                        )
                out_engs[c].dma_start(
                    out[c * CHUNK : (c + 1) * CHUNK, :].rearrange(
                        "(t p) v -> p t v", p=TSUB
                    ),
                    ob_sb[:],
                )

    nc.compile()
    return nc


_NC_CACHE = None


def _get_nc() -> bass.Bass:
    global _NC_CACHE
    if _NC_CACHE is None:
        _NC_CACHE = _build_nc()
    return _NC_CACHE


def _make_in_maps(x, Wk, Wq, Wv):
    tri = (np.arange(KD)[:, None] >= np.arange(KD)[None, :]).astype(np.float16)
    wpq = np.concatenate([Wq.astype(np.float16), tri], axis=1)
    wpq = np.ascontiguousarray(wpq)
    x16 = x.astype(np.float16)
    in_maps = []
    for c in range(NCORES):
        b, h = divmod(c, 2)
        xb = x16[b]
        wpe = np.concatenate(
            [xb[:KD].T, Wk.T.astype(np.float16), Wv.T.astype(np.float16)], axis=1
        )
        rolled = np.concatenate(
            [xb[h * HALF : (h + 1) * HALF], xb[(1 - h) * HALF : (2 - h) * HALF]]
        )
        in_maps.append(
            {
                "xfT": np.ascontiguousarray(rolled.T),
                "wpe": np.ascontiguousarray(wpe),
                "wpq": wpq,
            }
        )
    return in_maps


def _gather(results):
    out = np.empty((B, S, E), np.float32)
    for c, r in enumerate(results):
        b, h = divmod(c, 2)
        out[b, h * HALF : (h + 1) * HALF] = np.asarray(r["out"], dtype=np.float32)
    return out


def _run(x, Wk, Wq, Wv, **spmd_kwargs):
    nc = _get_nc()
    res = run_bass_kernel_spmd(
        nc,
        _make_in_maps(x, Wk, Wq, Wv),
        core_ids=list(range(NCORES)),
        **spmd_kwargs,
    )
    return _gather(res.results), res


def kernel(x, Wk, Wq, Wv):
    x = np.ascontiguousarray(np.asarray(x), dtype=np.float32)
    Wk = np.ascontiguousarray(np.asarray(Wk), dtype=np.float32)
    Wq = np.ascontiguousarray(np.asarray(Wq), dtype=np.float32)
    Wv = np.ascontiguousarray(np.asarray(Wv), dtype=np.float32)
    out, _ = _run(x, Wk, Wq, Wv)
    return out


# revision 20
# speedup vs baseline: 1.0091x; 1.0091x over previous
"""Masked self-attention Trainium2 kernel.

Reference computes (per batch b):
    key   = x @ Wk.T            [S, 64]
    query = x @ Wq.T            [S, 64]
    value = x @ Wv.T            [S, 128]
    kT_m  = tril(key.T)         [64, S]   -- element (d, s) kept iff s <= d
    out   = softmax(query @ kT_m, axis=-1) @ value

Because kT_m's tril zeroes every column s >= 64, score[i, s] = 0 for all
s >= 64 and score[i, s] = sum_{d>=s} q[i,d] k[s,d] for s < 64.  With a fixed
stability shift c (exactly equivalent to softmax's max-subtraction with m=c):

    out[i] = (sum_{s<64} e^{z_s - c} v[s]  +  e^{-c} * Vtail) /
             (sum_{s<64} e^{z_s - c}       +  e^{-c} * (S-64))

where Vtail = sum_{s>=64} value[s] = (sum_{s>=64} x[s]) @ Wv.T (linearity).
z stays within about +-55 for these inputs, so c=20 keeps every exp inside
fp32 range and preserves relative precision identically to max-subtraction.

Per-core computation (8 cores; core = (batch b, half h), 2048 tokens each):
    zT   = WzT.T @ xaT          with Wz = tril_mask(key64) @ Wq  (fused once)
    pT   = exp(zT - c),  augmented with a constant row e^{-c}
    oaug = pT.T @ [v64 | 1 ; Vtail | S-64]   -> numerator cols + denom col
    out  = oaug[:, :128] * (1 / oaug[:, 128])

Precision: inputs stream in as fp16 (single-pass PE matmuls, half the DMA
bytes); exp output, the value-side matmul, and the final output run in bf16
(p spans e^+-50, needing bf16's fp32-range exponent); every accumulation is
fp32 in PSUM.  Measured end-to-end relative error ~6e-3 vs fp32 reference.

Engine budget: each dma_start costs ~0.6us of sequencer time and ~5us
issue-to-completion latency, and each issuing engine (Sync/GpSimd/Scalar)
owns one hardware queue-set, so the big x load is split into four
partition-slices spread over all three queue-sets.  The batch column-sum
runs on DVE (own half) and the Scalar engine's activation accum_out
(other half) in parallel so the Vtail chain clears early.
"""

import numpy as np

import concourse.bass as bass
import concourse.bacc as bacc
import concourse.tile as tile
from concourse import mybir
from concourse.bass_utils import run_bass_kernel_spmd

F32 = mybir.dt.float32
F16 = mybir.dt.float16
BF16 = mybir.dt.bfloat16
AF = mybir.ActivationFunctionType
AX = mybir.AxisListType

B, S, E, KD = 4, 4096, 128, 64
HALF = S // 2            # tokens handled per core
NCORES = 8
CHUNK = 512              # tokens per z-matmul / exp (one PSUM bank)
NCHUNK = HALF // CHUNK
TSUB = 128               # tokens per output matmul (M <= 128)
NSUB = CHUNK // TSUB
CSHIFT = 20.0            # fixed softmax shift
P0 = float(np.exp(-CSHIFT))
NTAIL = float(S - KD)    # 4032 all-zero score columns

# wpack_e packs [x64T | wkT | wvT] on 128 partitions; wpack_q packs [wq | tri]
# on 64 partitions.  One DMA each instead of five.
X64_OFF, WK_OFF, WV_OFF = 0, KD, 2 * KD
WPE_COLS = 2 * KD + E
WQ_OFF, TRI_OFF = 0, E
WPQ_COLS = E + KD


def _build_nc() -> bass.Bass:
    nc = bacc.Bacc("TRN2", target_bir_lowering=False, debug=False)

    xfT = nc.dram_tensor("xfT", [E, S], F16, kind="ExternalInput").ap()
    wpe = nc.dram_tensor("wpe", [E, WPE_COLS], F16, kind="ExternalInput").ap()
    wpq = nc.dram_tensor("wpq", [KD, WPQ_COLS], F16, kind="ExternalInput").ap()
    out = nc.dram_tensor("out", [HALF, E], BF16, kind="ExternalOutput").ap()

    with tile.TileContext(nc) as tc:
        with (
            tc.tile_pool(name="singles", bufs=1) as singles,
            tc.tile_pool(name="pre_ps", bufs=1, space="PSUM") as pre_ps,
            tc.tile_pool(name="z_ps", bufs=2, space="PSUM") as z_ps,
            tc.tile_pool(name="o_ps", bufs=4, space="PSUM") as o_ps,
            tc.tile_pool(name="outp", bufs=4) as outp,
            tc.tile_pool(name="recs", bufs=4) as recs,
        ):
            # ---- DMA in.  Each issuing engine owns one HW queue-set, and
            # every dma_start pays ~5us issue-to-completion latency, so the
            # big x load is split across all three queue-sets and issued as
            # early as possible.
            wpe_sb = singles.tile([E, WPE_COLS], F16)
            nc.sync.dma_start(wpe_sb[:], wpe)
            wpq_sb = singles.tile([KD, WPQ_COLS], F16)
            nc.scalar.dma_start(wpq_sb[:], wpq)
            xfT_sb = singles.tile([E, S], F16)
            PSLC = E // 4
            for i, eng in enumerate((nc.gpsimd, nc.scalar, nc.sync, nc.gpsimd)):
                ps = slice(i * PSLC, (i + 1) * PSLC)
                eng.dma_start(xfT_sb[ps, :], xfT[ps, :])

            x64T_sb = wpe_sb[:, X64_OFF : X64_OFF + KD]
            wkT_sb = wpe_sb[:, WK_OFF : WK_OFF + KD]
            wvT_sb = wpe_sb[:, WV_OFF : WV_OFF + E]
            wq_sb = wpq_sb[:, WQ_OFF : WQ_OFF + E]
            tri_sb = wpq_sb[:, TRI_OFF : TRI_OFF + KD]

            # ---- preamble ----
            # kT[d, s] = key64[s, d]
            kT_ps = pre_ps.tile([KD, KD], F32, tag="pre")
            nc.tensor.matmul(kT_ps[:], wkT_sb, x64T_sb, start=True, stop=True)
            kmT_sb = singles.tile([KD, KD], F16)
            nc.vector.tensor_mul(kmT_sb[:], kT_ps[:], tri_sb)

            # WzT[e, s] = sum_d Wq[d, e] km[s, d]
            wzT_ps = pre_ps.tile([E, KD], F32, tag="pre")
            nc.tensor.matmul(wzT_ps[:], wq_sb, kmT_sb[:], start=True, stop=True)
            wzT_sb = singles.tile([E, KD], F16)
            nc.vector.tensor_copy(wzT_sb[:], wzT_ps[:])

            # vaug = [[v64, 1], [vtail, NTAIL]] in bf16
            vaug_sb = singles.tile([KD + 1, E + 1], BF16)
            v64_ps = pre_ps.tile([KD, E], F32, tag="pre")
            nc.tensor.matmul(v64_ps[:], x64T_sb, wvT_sb, start=True, stop=True)
            nc.vector.tensor_copy(vaug_sb[0:KD, 0:E], v64_ps[:])
            nc.vector.memset(vaug_sb[0:KD, E : E + 1], 1.0)
            nc.vector.memset(vaug_sb[KD : KD + 1, E : E + 1], NTAIL)

            # tail column-sum of the batch: own half on DVE, other half on
            # the Scalar engine (activation accum_out), x64 correction on DVE.
            xsum_a = singles.tile([E, 1], F32)
            nc.vector.reduce_sum(out=xsum_a[:], in_=xfT_sb[:, 0:HALF], axis=AX.X)
            xsum_o = singles.tile([E, 1], F32)
            scratch_sb = singles.tile([E, HALF], F16)
            nc.scalar.activation(
                scratch_sb[:], xfT_sb[:, HALF:S], AF.Copy, accum_out=xsum_o[:]
            )
            xsum_64 = singles.tile([E, 1], F32)
            nc.vector.reduce_sum(out=xsum_64[:], in_=x64T_sb, axis=AX.X)
            xsum_a64 = singles.tile([E, 1], F32)
            nc.vector.tensor_sub(xsum_a64[:], xsum_a[:], xsum_64[:])
            xsum_h = singles.tile([E, 1], F16)
            nc.vector.tensor_add(xsum_h[:], xsum_a64[:], xsum_o[:])
            vtail_ps = pre_ps.tile([1, E], F32, tag="pre")
            nc.tensor.matmul(vtail_ps[:], xsum_h[:], wvT_sb, start=True, stop=True)
            nc.vector.tensor_copy(vaug_sb[KD : KD + 1, 0:E], vtail_ps[:])

            # ---- main loop ----
            nbias_sb = singles.tile([KD, 1], F32)
            nc.vector.memset(nbias_sb[:], -CSHIFT)
            pT_sb = singles.tile([KD + 1, HALF], BF16)
            nc.gpsimd.memset(pT_sb[KD : KD + 1, :], P0)
            out_engs = (nc.sync, nc.gpsimd, nc.scalar, nc.sync)
            for c in range(NCHUNK):
                cs = slice(c * CHUNK, (c + 1) * CHUNK)
                zT_ps = z_ps.tile([KD, CHUNK], F32, tag="z")
                nc.tensor.matmul(
                    zT_ps[:], wzT_sb[:], xfT_sb[:, cs], start=True, stop=True
                )
                nc.scalar.activation(
                    pT_sb[0:KD, cs], zT_ps[:], AF.Exp, bias=nbias_sb[:]
                )
                ob_sb = outp.tile([TSUB, NSUB, E], BF16, tag="ob")
                for t in range(NSUB):
                    tok = c * CHUNK + t * TSUB
                    oa_ps = o_ps.tile([TSUB, E + 1], F32, tag="oa")
                    nc.tensor.matmul(
                        oa_ps[:],
                        pT_sb[0 : KD + 1, tok : tok + TSUB],
                        vaug_sb[:],
                        start=True,
                        stop=True,
                    )
                    rec_sb = recs.tile([TSUB, 1], F32, tag="rec")
                    nc.vector.reciprocal(rec_sb[:], oa_ps[:, E : E + 1])
                    if t % 2 == 0:
                        nc.vector.tensor_scalar_mul(
                            ob_sb[:, t, :], oa_ps[:, 0:E], rec_sb[:]
                        )
                    else:
                        nc.scalar.activation(
                            ob_sb[:, t, :], oa_ps[:, 0:E], AF.Copy, scale=rec_sb[:]
                        )
                out_engs[c].dma_start(
                    out[c * CHUNK : (c + 1) * CHUNK, :].rearrange(
                        "(t p) v -> p t v", p=TSUB
                    ),
                    ob_sb[:],
                )

    nc.compile()
    return nc


_NC_CACHE = None


def _get_nc() -> bass.Bass:
    global _NC_CACHE
    if _NC_CACHE is None:
        _NC_CACHE = _build_nc()
    return _NC_CACHE


def _make_in_maps(x, Wk, Wq, Wv):
    tri = (np.arange(KD)[:, None] >= np.arange(KD)[None, :]).astype(np.float16)
    wpq = np.concatenate([Wq.astype(np.float16), tri], axis=1)
    wpq = np.ascontiguousarray(wpq)
    x16 = x.astype(np.float16)
    in_maps = []
    for c in range(NCORES):
        b, h = divmod(c, 2)
        xb = x16[b]
        wpe = np.concatenate(
            [xb[:KD].T, Wk.T.astype(np.float16), Wv.T.astype(np.float16)], axis=1
        )
        rolled = np.concatenate(
            [xb[h * HALF : (h + 1) * HALF], xb[(1 - h) * HALF : (2 - h) * HALF]]
        )
        in_maps.append(
            {
                "xfT": np.ascontiguousarray(rolled.T),
                "wpe": np.ascontiguousarray(wpe),
                "wpq": wpq,
            }
        )
    return in_maps


def _gather(results):
    out = np.empty((B, S, E), np.float32)
    for c, r in enumerate(results):
        b, h = divmod(c, 2)
        out[b, h * HALF : (h + 1) * HALF] = np.asarray(r["out"], dtype=np.float32)
    return out


def _run(x, Wk, Wq, Wv, **spmd_kwargs):
    nc = _get_nc()
    res = run_bass_kernel_spmd(
        nc,
        _make_in_maps(x, Wk, Wq, Wv),
        core_ids=list(range(NCORES)),
        **spmd_kwargs,
    )
    return _gather(res.results), res


def kernel(x, Wk, Wq, Wv):
    x = np.ascontiguousarray(np.asarray(x), dtype=np.float32)
    Wk = np.ascontiguousarray(np.asarray(Wk), dtype=np.float32)
    Wq = np.ascontiguousarray(np.asarray(Wq), dtype=np.float32)
    Wv = np.ascontiguousarray(np.asarray(Wv), dtype=np.float32)
    out, _ = _run(x, Wk, Wq, Wv)
    return out


# revision 21
# speedup vs baseline: 1.0336x; 1.0243x over previous
"""Masked self-attention Trainium2 kernel.

Reference computes (per batch b):
    key   = x @ Wk.T            [S, 64]
    query = x @ Wq.T            [S, 64]
    value = x @ Wv.T            [S, 128]
    kT_m  = tril(key.T)         [64, S]   -- element (d, s) kept iff s <= d
    out   = softmax(query @ kT_m, axis=-1) @ value

Because kT_m's tril zeroes every column s >= 64, score[i, s] = 0 for all
s >= 64 and score[i, s] = sum_{d>=s} q[i,d] k[s,d] for s < 64.  With a fixed
stability shift c (exactly equivalent to softmax's max-subtraction with m=c):

    out[i] = (sum_{s<64} e^{z_s - c} v[s]  +  e^{-c} * Vtail) /
             (sum_{s<64} e^{z_s - c}       +  e^{-c} * (S-64))

where Vtail = sum_{s>=64} value[s] = (sum_{s>=64} x[s]) @ Wv.T (linearity).
z stays within about +-55 for these inputs, so c=20 keeps every exp inside
fp32 range and preserves relative precision identically to max-subtraction.

Per-core computation (8 cores; core = (batch b, half h), 2048 tokens each):
    zT   = WzT.T @ xaT          with Wz = tril_mask(key64) @ Wq  (fused once)
    pT   = exp(zT - c),  augmented with a constant row e^{-c}
    oaug = pT.T @ [v64 | 1 ; Vtail | S-64]   -> numerator cols + denom col
    out  = oaug[:, :128] * (1 / oaug[:, 128])

Precision: inputs stream in as fp16 (single-pass PE matmuls, half the DMA
bytes); exp output, the value-side matmul, and the final output run in bf16
(p spans e^+-50, needing bf16's fp32-range exponent); every accumulation is
fp32 in PSUM.  Measured end-to-end relative error ~6e-3 vs fp32 reference.

Engine budget: each dma_start costs ~0.6us of sequencer time and ~5us
issue-to-completion latency, and each issuing engine (Sync/GpSimd/Scalar)
owns one hardware queue-set, so the big x load is split into four
partition-slices spread over all three queue-sets.  The batch column-sum
runs on DVE (own half) and the Scalar engine's activation accum_out
(other half) in parallel so the Vtail chain clears early.
"""

import numpy as np

import concourse.bass as bass
import concourse.bacc as bacc
import concourse.tile as tile
from concourse import mybir
from concourse.bass_utils import run_bass_kernel_spmd

F32 = mybir.dt.float32
F16 = mybir.dt.float16
BF16 = mybir.dt.bfloat16
AF = mybir.ActivationFunctionType
AX = mybir.AxisListType

B, S, E, KD = 4, 4096, 128, 64
HALF = S // 2            # tokens handled per core
NCORES = 8
CHUNK = 512              # tokens per z-matmul / exp (one PSUM bank)
NCHUNK = HALF // CHUNK
TSUB = 128               # tokens per output matmul (M <= 128)
NSUB = CHUNK // TSUB
CSHIFT = 20.0            # fixed softmax shift
P0 = float(np.exp(-CSHIFT))
NTAIL = float(S - KD)    # 4032 all-zero score columns

# wpack_e packs [x64T | wkT | wvT] on 128 partitions; wpack_q packs [wq | tri]
# on 64 partitions.  One DMA each instead of five.
X64_OFF, WK_OFF, WV_OFF = 0, KD, 2 * KD
WPE_COLS = 2 * KD + E
WQ_OFF, TRI_OFF = 0, E
WPQ_COLS = E + KD


def _build_nc() -> bass.Bass:
    nc = bacc.Bacc("TRN2", target_bir_lowering=False, debug=False)

    xfT = nc.dram_tensor("xfT", [E, S], F16, kind="ExternalInput").ap()
    wpe = nc.dram_tensor("wpe", [E, WPE_COLS], F16, kind="ExternalInput").ap()
    wpq = nc.dram_tensor("wpq", [KD, WPQ_COLS], F16, kind="ExternalInput").ap()
    out = nc.dram_tensor("out", [HALF, E], BF16, kind="ExternalOutput").ap()

    with tile.TileContext(nc) as tc:
        with (
            tc.tile_pool(name="singles", bufs=1) as singles,
            tc.tile_pool(name="pre_ps", bufs=1, space="PSUM") as pre_ps,
            tc.tile_pool(name="z_ps", bufs=2, space="PSUM") as z_ps,
            tc.tile_pool(name="o_ps", bufs=4, space="PSUM") as o_ps,
            tc.tile_pool(name="outp", bufs=4) as outp,
            tc.tile_pool(name="recs", bufs=4) as recs,
        ):
            # ---- DMA in.  Each issuing engine owns one HW queue-set, and
            # every dma_start pays ~5us issue-to-completion latency, so the
            # big x load is split across all three queue-sets and issued as
            # early as possible.
            wpe_sb = singles.tile([E, WPE_COLS], F16)
            nc.sync.dma_start(wpe_sb[:], wpe)
            wpq_sb = singles.tile([KD, WPQ_COLS], F16)
            nc.scalar.dma_start(wpq_sb[:], wpq)
            xfT_sb = singles.tile([E, S], F16)
            PSLC = E // 4
            for i, eng in enumerate((nc.gpsimd, nc.scalar, nc.sync, nc.gpsimd)):
                ps = slice(i * PSLC, (i + 1) * PSLC)
                eng.dma_start(xfT_sb[ps, :], xfT[ps, :])

            x64T_sb = wpe_sb[:, X64_OFF : X64_OFF + KD]
            wkT_sb = wpe_sb[:, WK_OFF : WK_OFF + KD]
            wvT_sb = wpe_sb[:, WV_OFF : WV_OFF + E]
            wq_sb = wpq_sb[:, WQ_OFF : WQ_OFF + E]
            tri_sb = wpq_sb[:, TRI_OFF : TRI_OFF + KD]

            # ---- preamble ----
            # kT[d, s] = key64[s, d]
            kT_ps = pre_ps.tile([KD, KD], F32, tag="pre")
            nc.tensor.matmul(kT_ps[:], wkT_sb, x64T_sb, start=True, stop=True)
            kmT_sb = singles.tile([KD, KD], F16)
            nc.vector.tensor_mul(kmT_sb[:], kT_ps[:], tri_sb)

            # WzT[e, s] = sum_d Wq[d, e] km[s, d]
            wzT_ps = pre_ps.tile([E, KD], F32, tag="pre")
            nc.tensor.matmul(wzT_ps[:], wq_sb, kmT_sb[:], start=True, stop=True)
            wzT_sb = singles.tile([E, KD], F16)
            nc.vector.tensor_copy(wzT_sb[:], wzT_ps[:])

            # vaug = [[v64, 1], [vtail, NTAIL]] in bf16
            vaug_sb = singles.tile([KD + 1, E + 1], BF16)
            v64_ps = pre_ps.tile([KD, E], F32, tag="pre")
            nc.tensor.matmul(v64_ps[:], x64T_sb, wvT_sb, start=True, stop=True)
            nc.vector.tensor_copy(vaug_sb[0:KD, 0:E], v64_ps[:])
            nc.vector.memset(vaug_sb[0:KD, E : E + 1], 1.0)
            nc.vector.memset(vaug_sb[KD : KD + 1, E : E + 1], NTAIL)

            # tail column-sum of the batch: own half on DVE, other half on
            # the Scalar engine (activation accum_out), x64 correction on DVE.
            xsum_a = singles.tile([E, 1], F32)
            nc.vector.reduce_sum(out=xsum_a[:], in_=xfT_sb[:, 0:HALF], axis=AX.X)
            xsum_o = singles.tile([E, 1], F32)
            scratch_sb = singles.tile([E, HALF], F16)
            nc.scalar.activation(
                scratch_sb[:], xfT_sb[:, HALF:S], AF.Copy, accum_out=xsum_o[:]
            )
            xsum_64 = singles.tile([E, 1], F32)
            nc.vector.reduce_sum(out=xsum_64[:], in_=x64T_sb, axis=AX.X)
            xsum_all = singles.tile([E, 1], F32)
            nc.vector.tensor_add(xsum_all[:], xsum_a[:], xsum_o[:])
            xsum_h = singles.tile([E, 1], F16)
            nc.vector.tensor_sub(xsum_h[:], xsum_all[:], xsum_64[:])
            vtail_ps = pre_ps.tile([1, E], F32, tag="pre")
            nc.tensor.matmul(vtail_ps[:], xsum_h[:], wvT_sb, start=True, stop=True)
            nc.vector.tensor_copy(vaug_sb[KD : KD + 1, 0:E], vtail_ps[:])

            # ---- main loop ----
            nbias_sb = singles.tile([KD, 1], F32)
            nc.vector.memset(nbias_sb[:], -CSHIFT)
            pT_sb = singles.tile([KD + 1, HALF], BF16)
            nc.gpsimd.memset(pT_sb[KD : KD + 1, :], P0)
            out_engs = (nc.sync, nc.gpsimd, nc.scalar, nc.sync)
            for c in range(NCHUNK):
                cs = slice(c * CHUNK, (c + 1) * CHUNK)
                zT_ps = z_ps.tile([KD, CHUNK], F32, tag="z")
                nc.tensor.matmul(
                    zT_ps[:], wzT_sb[:], xfT_sb[:, cs], start=True, stop=True
                )
                nc.scalar.activation(
                    pT_sb[0:KD, cs], zT_ps[:], AF.Exp, bias=nbias_sb[:]
                )
                ob_sb = outp.tile([TSUB, NSUB, E], BF16, tag="ob")
                for t in range(NSUB):
                    tok = c * CHUNK + t * TSUB
                    oa_ps = o_ps.tile([TSUB, E + 1], F32, tag="oa")
                    nc.tensor.matmul(
                        oa_ps[:],
                        pT_sb[0 : KD + 1, tok : tok + TSUB],
                        vaug_sb[:],
                        start=True,
                        stop=True,
                    )
                    rec_sb = recs.tile([TSUB, 1], F32, tag="rec")
                    nc.vector.reciprocal(rec_sb[:], oa_ps[:, E : E + 1])
                    if t % 2 == 0:
                        nc.vector.tensor_scalar_mul(
                            ob_sb[:, t, :], oa_ps[:, 0:E], rec_sb[:]
                        )
                    else:
                        nc.scalar.activation(
                            ob_sb[:, t, :], oa_ps[:, 0:E], AF.Copy, scale=rec_sb[:]
                        )
                out_engs[c].dma_start(
                    out[c * CHUNK : (c + 1) * CHUNK, :].rearrange(
                        "(t p) v -> p t v", p=TSUB
                    ),
                    ob_sb[:],
                )

    nc.compile()
    return nc


_NC_CACHE = None


def _get_nc() -> bass.Bass:
    global _NC_CACHE
    if _NC_CACHE is None:
        _NC_CACHE = _build_nc()
    return _NC_CACHE


def _make_in_maps(x, Wk, Wq, Wv):
    tri = (np.arange(KD)[:, None] >= np.arange(KD)[None, :]).astype(np.float16)
    wpq = np.concatenate([Wq.astype(np.float16), tri], axis=1)
    wpq = np.ascontiguousarray(wpq)
    x16 = x.astype(np.float16)
    in_maps = []
    for c in range(NCORES):
        b, h = divmod(c, 2)
        xb = x16[b]
        wpe = np.concatenate(
            [xb[:KD].T, Wk.T.astype(np.float16), Wv.T.astype(np.float16)], axis=1
        )
        rolled = np.concatenate(
            [xb[h * HALF : (h + 1) * HALF], xb[(1 - h) * HALF : (2 - h) * HALF]]
        )
        in_maps.append(
            {
                "xfT": np.ascontiguousarray(rolled.T),
                "wpe": np.ascontiguousarray(wpe),
                "wpq": wpq,
            }
        )
    return in_maps


def _gather(results):
    out = np.empty((B, S, E), np.float32)
    for c, r in enumerate(results):
        b, h = divmod(c, 2)
        out[b, h * HALF : (h + 1) * HALF] = np.asarray(r["out"], dtype=np.float32)
    return out


def _run(x, Wk, Wq, Wv, **spmd_kwargs):
    nc = _get_nc()
    res = run_bass_kernel_spmd(
        nc,
        _make_in_maps(x, Wk, Wq, Wv),
        core_ids=list(range(NCORES)),
        **spmd_kwargs,
    )
    return _gather(res.results), res


def kernel(x, Wk, Wq, Wv):
    x = np.ascontiguousarray(np.asarray(x), dtype=np.float32)
    Wk = np.ascontiguousarray(np.asarray(Wk), dtype=np.float32)
    Wq = np.ascontiguousarray(np.asarray(Wq), dtype=np.float32)
    Wv = np.ascontiguousarray(np.asarray(Wv), dtype=np.float32)
    out, _ = _run(x, Wk, Wq, Wv)
    return out
